# revision 2
# baseline (speedup 1.0000x reference)
"""Trainium2 Bass kernel for nn_ANO_VQC_Model (14-qubit VQC, batch 512).

Math: the circuit's state, viewed as a 128x128 matrix M (rows = qubits 0-6,
cols = qubits 7-13), starts as a real rank-1 outer product u v^T (RY layer on
|+>^14 gives a real product state) and each entangling layer acts as
    M' = A0 M B0^T + A1 M B1^T
(only CNOT(6,7) couples rows and cols; it splits into 2 terms via projectors
on qubit 6).  So the state stays factored: L <- [A0 L | A1 L],
R <- [B0 R | B1 R], M = L R^T with rank <= 64 after 6 layers.  Everything is
real f32.  The two requested expectation values are
    e_q = sum( (L^T G_q L) * (R^T R) ),  G_q = Re(H_q) (x) I_16  (row space).

Sharding: pure data parallel, 64 batch elements per core on 8 cores.
"""

import os
import sys

import numpy as np

for _p in ("/opt/trn_rl_repo", "/root/.axon_site/_ro/trn_rl_repo"):
    if os.path.isdir(_p) and _p not in sys.path:
        sys.path.append(_p)

import concourse.bass as bass
import concourse.mybir as mybir
import concourse.tile as tile
from concourse import bacc
from concourse.bass_utils import run_bass_kernel_spmd

N_CORES = 8
BATCH = 512
BPC = BATCH // N_CORES  # 64
NQ = 14
DEPTH = 6
DA = 128  # row space (qubits 0-6)
DB = 128  # col space (qubits 7-13)

F32 = mybir.dt.float32
# dtype used for the matmul input tensors (weights / L / R / P buffers)
MM_DT = mybir.dt.float32

_nc_cache = {}


# ----------------------------------------------------------------------------
# Host-side preprocessing (input-dependent constant folding)
# ----------------------------------------------------------------------------

def _ry(theta):
    c, s = np.cos(theta / 2), np.sin(theta / 2)
    return np.array([[c, -s], [s, c]], dtype=np.float64)


_CNOT = np.array(
    [[1, 0, 0, 0], [0, 1, 0, 0], [0, 0, 0, 1], [0, 0, 1, 0]], dtype=np.float64
)


def _kron_list(mats):
    out = mats[0]
    for m in mats[1:]:
        out = np.kron(out, m)
    return out


def _cnot_on(n, ctrl):
    mats, q = [], 0
    while q < n:
        if q == ctrl:
            mats.append(_CNOT)
            q += 2
        else:
            mats.append(np.eye(2))
            q += 1
    return _kron_list(mats)


def _layer_mats(theta_k):
    """A0, A1 (row ops) and B0, B1 (col ops) for one entangling layer."""
    C_evenA = _cnot_on(7, 0) @ _cnot_on(7, 2) @ _cnot_on(7, 4)
    C_oddA = _cnot_on(7, 1) @ _cnot_on(7, 3) @ _cnot_on(7, 5)
    R_A = _kron_list([_ry(theta_k[w]) for w in range(7)])
    C_evenB = _cnot_on(7, 1) @ _cnot_on(7, 3) @ _cnot_on(7, 5)
    C_oddB = _cnot_on(7, 0) @ _cnot_on(7, 2) @ _cnot_on(7, 4)
    R_B = _kron_list([_ry(theta_k[7 + w]) for w in range(7)])
    rows = np.arange(DA)
    P0 = np.diag((rows % 2 == 0).astype(np.float64))
    P1 = np.diag((rows % 2 == 1).astype(np.float64))
    S = np.zeros((DB, DB))
    S[: DB // 2, DB // 2:] = np.eye(DB // 2)
    S[DB // 2:, : DB // 2] = np.eye(DB // 2)
    A0 = R_A @ C_oddA @ P0 @ C_evenA
    A1 = R_A @ C_oddA @ P1 @ C_evenA
    B0 = R_B @ C_oddB @ C_evenB
    B1 = R_B @ C_oddB @ S @ C_evenB
    return A0, A1, B0, B1


def _measure_mats(A, B, D):
    """G_q = Re(H_q) expanded to the 128-dim row space, q = 0, 1."""
    NLOC = 8
    rows_t, cols_t = np.tril_indices(NLOC, -1)
    Gs = []
    for q in range(2):
        tri = np.zeros((NLOC, NLOC))
        tri[rows_t, cols_t] = A[q]
        h = tri + np.diag(np.concatenate([D[q][1:], [0.0]]))
        Hr = h + h.T
        if q == 0:
            G = np.kron(Hr, np.eye(16))  # wires 0,1,2 -> row bits 0-2
        else:
            G = np.kron(np.kron(np.eye(2), Hr), np.eye(8))  # wires 1,2,3
        Gs.append(G)
    return np.stack(Gs)


def _host_prep(X, theta, A, B, D):
    X = np.asarray(X, dtype=np.float64)
    theta = np.asarray(theta, dtype=np.float64)
    A = np.asarray(A, dtype=np.float64)
    B = np.asarray(B, dtype=np.float64)
    D = np.asarray(D, dtype=np.float64)
    nb = X.shape[0]
    c, s = np.cos(X / 2), np.sin(X / 2)
    v0 = (c - s) / np.sqrt(2.0)
    v1 = (c + s) / np.sqrt(2.0)

    def kron_side(ws):
        out = np.ones((nb, 1))
        for w in ws:
            pair = np.stack([v0[:, w], v1[:, w]], axis=1)
            out = (out[:, :, None] * pair[:, None, :]).reshape(nb, -1)
        return out

    U = kron_side(range(7))  # (B, 128)
    V = kron_side(range(7, 14))
    AT = np.empty((2 * DEPTH, DA, DA))
    BT = np.empty((2 * DEPTH, DB, DB))
    for k in range(DEPTH):
        A0, A1, B0, B1 = _layer_mats(theta[k])
        AT[2 * k + 0] = A0.T  # lhsT layout: out = lhsT.T @ rhs
        AT[2 * k + 1] = A1.T
        BT[2 * k + 0] = B0.T
        BT[2 * k + 1] = B1.T
    G = _measure_mats(A, B, D)  # (2, 128, 128), symmetric
    return U, V, AT, BT, G


# ----------------------------------------------------------------------------
# Device kernel
# ----------------------------------------------------------------------------

def _build_nc():
    nc = bacc.Bacc("TRN2", target_bir_lowering=False, debug=False)

    ut_d = nc.declare_dram_parameter("ut", [DA, BPC], MM_DT, isOutput=False)
    vt_d = nc.declare_dram_parameter("vt", [DB, BPC], MM_DT, isOutput=False)
    at_d = nc.declare_dram_parameter("at", [2 * DEPTH, DA, DA], MM_DT, isOutput=False)
    bt_d = nc.declare_dram_parameter("bt", [2 * DEPTH, DB, DB], MM_DT, isOutput=False)
    g_d = nc.declare_dram_parameter("g", [2, DA, DA], MM_DT, isOutput=False)
    out_d = nc.declare_dram_parameter("out", [1, 2 * BPC], F32, isOutput=True)

    with tile.TileContext(nc) as tc:
        with (
            tc.tile_pool(name="w", bufs=1) as wpool,
            tc.tile_pool(name="state", bufs=1) as spool,
            tc.tile_pool(name="grp", bufs=2) as gpool,
            tc.tile_pool(name="ps", bufs=2, space="PSUM") as pspool,
        ):
            aw = wpool.tile([DA, 2 * DEPTH * DA], MM_DT, tag="aw")
            bw = wpool.tile([DB, 2 * DEPTH * DB], MM_DT, tag="bw")
            gw = wpool.tile([DA, 2 * DA], MM_DT, tag="gw")
            ut = wpool.tile([DA, BPC], MM_DT, tag="ut")
            vt = wpool.tile([DB, BPC], MM_DT, tag="vt")
            ones = wpool.tile([64, 1], F32, tag="ones")

            for i in range(2 * DEPTH):
                nc.sync.dma_start(out=aw[:, i * DA:(i + 1) * DA], in_=at_d[i])
                nc.sync.dma_start(out=bw[:, i * DB:(i + 1) * DB], in_=bt_d[i])
            nc.sync.dma_start(out=gw[:, 0:DA], in_=g_d[0])
            nc.sync.dma_start(out=gw[:, DA:2 * DA], in_=g_d[1])
            nc.sync.dma_start(out=ut[:], in_=ut_d[:, :])
            nc.sync.dma_start(out=vt[:], in_=vt_d[:, :])
            nc.vector.memset(ones[:], 1.0)

            Ltmp = spool.tile([DA, 32 * BPC], MM_DT, tag="Ltmp")
            Lbuf = spool.tile([DA, 64 * BPC], MM_DT, tag="Lbuf")
            Rtmp = spool.tile([DB, 32 * BPC], MM_DT, tag="Rtmp")
            Rbuf = spool.tile([DB, 64 * BPC], MM_DT, tag="Rbuf")
            Pbuf = spool.tile([DA, 2 * 64 * BPC], MM_DT, tag="Pbuf")
            Z = spool.tile([64, 2 * BPC], F32, tag="Z")
            esb = spool.tile([1, 2 * BPC], F32, tag="esb")

            def recursion(w_tile, init_ap, buf_tmp, buf_big):
                cur, n_in = init_ap, BPC
                for k in range(DEPTH):
                    dst = buf_tmp if k % 2 == 0 else buf_big
                    for p in range(2):
                        lhsT = w_tile[:, (2 * k + p) * 128:(2 * k + p + 1) * 128]
                        for c0 in range(0, n_in, 512):
                            cw = min(512, n_in - c0)
                            ps = pspool.tile([128, 512], F32, tag="mm")
                            nc.tensor.matmul(
                                ps[:, :cw], lhsT, cur[:, c0:c0 + cw],
                                start=True, stop=True,
                            )
                            nc.vector.tensor_copy(
                                dst[:, p * n_in + c0:p * n_in + c0 + cw],
                                ps[:, :cw],
                            )
                    cur, n_in = dst[:, :2 * n_in], 2 * n_in
                return cur  # (128, 64*BPC)

            Lfin = recursion(aw, ut[:], Ltmp, Lbuf)
            Rfin = recursion(bw, vt[:], Rtmp, Rbuf)

            # P = [G0 @ L | G1 @ L]  -> (128, (q, j, b))
            NL = 64 * BPC  # 4096
            for q in range(2):
                for c0 in range(0, NL, 512):
                    ps = pspool.tile([128, 512], F32, tag="mm")
                    nc.tensor.matmul(
                        ps[:], gw[:, q * DA:(q + 1) * DA], Lfin[:, c0:c0 + 512],
                        start=True, stop=True,
                    )
                    nc.vector.tensor_copy(Pbuf[:, q * NL + c0:q * NL + c0 + 512], ps[:])

            Lr = Lfin.rearrange("p (j b) -> p j b", b=BPC)
            Rr = Rfin.rearrange("p (j b) -> p j b", b=BPC)
            Pr = Pbuf[:].rearrange("p (q j b) -> p q j b", q=2, b=BPC)

            # Per-batch quadratic forms, 8 batches per group.
            GRP = 8
            n_groups = BPC // GRP
            for g in range(n_groups):
                slg = pspool.tile([64, GRP * 128], F32, tag="slg")
                srg = pspool.tile([64, GRP * 64], F32, tag="srg")
                for i in range(GRP):
                    b = g * GRP + i
                    nc.tensor.matmul(
                        slg[:, i * 128:(i + 1) * 128], Lr[:, :, b], Pr[:, :, :, b],
                        start=True, stop=True,
                    )
                    nc.tensor.matmul(
                        srg[:, i * 64:(i + 1) * 64], Rr[:, :, b], Rr[:, :, b],
                        start=True, stop=True,
                    )
                srsb = gpool.tile([64, GRP * 64], F32, tag="srsb")
                nc.vector.tensor_copy(srsb[:], srg[:])
                slg_r = slg[:].rearrange("p (i q j) -> p i q j", q=2, j=64)
                srsb_r = srsb[:].rearrange("p (i j) -> p i j", j=64)
                t0 = gpool.tile([64, GRP * 64], F32, tag="t0")
                t1 = gpool.tile([64, GRP * 64], F32, tag="t1")
                t0_r = t0[:].rearrange("p (i j) -> p i j", j=64)
                t1_r = t1[:].rearrange("p (i j) -> p i j", j=64)
                nc.vector.tensor_mul(t0_r, slg_r[:, :, 0, :], srsb_r)
                nc.vector.tensor_mul(t1_r, slg_r[:, :, 1, :], srsb_r)
                nc.vector.reduce_sum(
                    out=Z[:, g * GRP:(g + 1) * GRP], in_=t0_r,
                    axis=mybir.AxisListType.X,
                )
                nc.vector.reduce_sum(
                    out=Z[:, BPC + g * GRP:BPC + (g + 1) * GRP], in_=t1_r,
                    axis=mybir.AxisListType.X,
                )

            # e[q*BPC + b] = sum over the 64 term-partitions
            zps = pspool.tile([1, 2 * BPC], F32, tag="mm")
            nc.tensor.matmul(zps[:], ones[:], Z[:], start=True, stop=True)
            nc.vector.tensor_copy(esb[:], zps[:])
            nc.sync.dma_start(out=out_d[:, :], in_=esb[:])

    nc.compile()
    return nc


def _get_nc():
    if "nc" not in _nc_cache:
        _nc_cache["nc"] = _build_nc()
    return _nc_cache["nc"]


# ----------------------------------------------------------------------------
# Entry point
# ----------------------------------------------------------------------------

def kernel(X, theta, A, B, D, _trace=False):
    U, V, AT, BT, G = _host_prep(X, theta, A, B, D)
    np_mm = np.float32 if MM_DT == mybir.dt.float32 else mybir.dt.np(MM_DT)
    at = np.ascontiguousarray(AT, dtype=np_mm)
    bt = np.ascontiguousarray(BT, dtype=np_mm)
    g = np.ascontiguousarray(G, dtype=np_mm)
    in_maps = []
    for i in range(N_CORES):
        sl = slice(i * BPC, (i + 1) * BPC)
        in_maps.append(
            {
                "ut": np.ascontiguousarray(U[sl].T, dtype=np_mm),
                "vt": np.ascontiguousarray(V[sl].T, dtype=np_mm),
                "at": at,
                "bt": bt,
                "g": g,
            }
        )
    nc = _get_nc()
    res = run_bass_kernel_spmd(nc, in_maps, list(range(N_CORES)), trace=_trace)
    outs = []
    for i in range(N_CORES):
        e = res.results[i]["out"].reshape(2, BPC).T  # (64, 2)
        outs.append(e)
    full = np.concatenate(outs, axis=0).astype(np.float32)
    if _trace:
        _nc_cache["last_exec_ns"] = res.exec_time_ns
        _nc_cache["last_results"] = res
    return full


# revision 3
# speedup vs baseline: 1.4906x; 1.4906x over previous
"""Trainium2 Bass kernel for nn_ANO_VQC_Model (14-qubit VQC, batch 512).

Math: the circuit's state, viewed as a 128x128 matrix M (rows = qubits 0-6,
cols = qubits 7-13), starts as a real rank-1 outer product u v^T (RY layer on
|+>^14 gives a real product state) and each entangling layer acts as
    M' = A0 M B0^T + A1 M B1^T
(only CNOT(6,7) couples rows and cols; it splits into 2 terms via projectors
on qubit 6).  So the state stays factored: L <- [A0 L | A1 L],
R <- [B0 R | B1 R], M = L R^T with rank <= 64 after 6 layers.  Everything is
real f32.  The two requested expectation values are
    e_q = sum( (L^T G_q L) * (R^T R) ),  G_q = Re(H_q) (x) I_16  (row space).

Sharding: pure data parallel, 64 batch elements per core on 8 cores.
"""

import os
import sys

import numpy as np

for _p in ("/opt/trn_rl_repo", "/root/.axon_site/_ro/trn_rl_repo"):
    if os.path.isdir(_p) and _p not in sys.path:
        sys.path.append(_p)

import concourse.bass as bass
import concourse.mybir as mybir
import concourse.tile as tile
from concourse import bacc
from concourse.bass_utils import run_bass_kernel_spmd

N_CORES = 8
BATCH = 512
BPC = BATCH // N_CORES  # 64
NQ = 14
DEPTH = 6
DA = 128  # row space (qubits 0-6)
DB = 128  # col space (qubits 7-13)

F32 = mybir.dt.float32
# dtype used for the matmul input tensors (weights / L / R / P buffers)
MM_DT = mybir.dt.bfloat16

_nc_cache = {}


# ----------------------------------------------------------------------------
# Host-side preprocessing (input-dependent constant folding)
# ----------------------------------------------------------------------------

def _ry(theta):
    c, s = np.cos(theta / 2), np.sin(theta / 2)
    return np.array([[c, -s], [s, c]], dtype=np.float64)


_CNOT = np.array(
    [[1, 0, 0, 0], [0, 1, 0, 0], [0, 0, 0, 1], [0, 0, 1, 0]], dtype=np.float64
)


def _kron_list(mats):
    out = mats[0]
    for m in mats[1:]:
        out = np.kron(out, m)
    return out


def _cnot_on(n, ctrl):
    mats, q = [], 0
    while q < n:
        if q == ctrl:
            mats.append(_CNOT)
            q += 2
        else:
            mats.append(np.eye(2))
            q += 1
    return _kron_list(mats)


def _layer_mats(theta_k):
    """A0, A1 (row ops) and B0, B1 (col ops) for one entangling layer."""
    C_evenA = _cnot_on(7, 0) @ _cnot_on(7, 2) @ _cnot_on(7, 4)
    C_oddA = _cnot_on(7, 1) @ _cnot_on(7, 3) @ _cnot_on(7, 5)
    R_A = _kron_list([_ry(theta_k[w]) for w in range(7)])
    C_evenB = _cnot_on(7, 1) @ _cnot_on(7, 3) @ _cnot_on(7, 5)
    C_oddB = _cnot_on(7, 0) @ _cnot_on(7, 2) @ _cnot_on(7, 4)
    R_B = _kron_list([_ry(theta_k[7 + w]) for w in range(7)])
    rows = np.arange(DA)
    P0 = np.diag((rows % 2 == 0).astype(np.float64))
    P1 = np.diag((rows % 2 == 1).astype(np.float64))
    S = np.zeros((DB, DB))
    S[: DB // 2, DB // 2:] = np.eye(DB // 2)
    S[DB // 2:, : DB // 2] = np.eye(DB // 2)
    A0 = R_A @ C_oddA @ P0 @ C_evenA
    A1 = R_A @ C_oddA @ P1 @ C_evenA
    B0 = R_B @ C_oddB @ C_evenB
    B1 = R_B @ C_oddB @ S @ C_evenB
    return A0, A1, B0, B1


def _measure_mats(A, B, D):
    """G_q = Re(H_q) expanded to the 128-dim row space, q = 0, 1."""
    NLOC = 8
    rows_t, cols_t = np.tril_indices(NLOC, -1)
    Gs = []
    for q in range(2):
        tri = np.zeros((NLOC, NLOC))
        tri[rows_t, cols_t] = A[q]
        h = tri + np.diag(np.concatenate([D[q][1:], [0.0]]))
        Hr = h + h.T
        if q == 0:
            G = np.kron(Hr, np.eye(16))  # wires 0,1,2 -> row bits 0-2
        else:
            G = np.kron(np.kron(np.eye(2), Hr), np.eye(8))  # wires 1,2,3
        Gs.append(G)
    return np.stack(Gs)


def _host_prep(X, theta, A, B, D):
    X = np.asarray(X, dtype=np.float64)
    theta = np.asarray(theta, dtype=np.float64)
    A = np.asarray(A, dtype=np.float64)
    B = np.asarray(B, dtype=np.float64)
    D = np.asarray(D, dtype=np.float64)
    nb = X.shape[0]
    c, s = np.cos(X / 2), np.sin(X / 2)
    v0 = (c - s) / np.sqrt(2.0)
    v1 = (c + s) / np.sqrt(2.0)

    def kron_side(ws):
        out = np.ones((nb, 1))
        for w in ws:
            pair = np.stack([v0[:, w], v1[:, w]], axis=1)
            out = (out[:, :, None] * pair[:, None, :]).reshape(nb, -1)
        return out

    U = kron_side(range(7))  # (B, 128)
    V = kron_side(range(7, 14))
    AT = np.empty((2 * DEPTH, DA, DA))
    BT = np.empty((2 * DEPTH, DB, DB))
    for k in range(DEPTH):
        A0, A1, B0, B1 = _layer_mats(theta[k])
        AT[2 * k + 0] = A0.T  # lhsT layout: out = lhsT.T @ rhs
        AT[2 * k + 1] = A1.T
        BT[2 * k + 0] = B0.T
        BT[2 * k + 1] = B1.T
    G = _measure_mats(A, B, D)  # (2, 128, 128), symmetric
    return U, V, AT, BT, G


# ----------------------------------------------------------------------------
# Device kernel
# ----------------------------------------------------------------------------

def _build_nc():
    nc = bacc.Bacc("TRN2", target_bir_lowering=False, debug=False)

    ut_d = nc.declare_dram_parameter("ut", [DA, BPC], MM_DT, isOutput=False)
    vt_d = nc.declare_dram_parameter("vt", [DB, BPC], MM_DT, isOutput=False)
    at_d = nc.declare_dram_parameter("at", [2 * DEPTH, DA, DA], MM_DT, isOutput=False)
    bt_d = nc.declare_dram_parameter("bt", [2 * DEPTH, DB, DB], MM_DT, isOutput=False)
    g_d = nc.declare_dram_parameter("g", [2, DA, DA], MM_DT, isOutput=False)
    out_d = nc.declare_dram_parameter("out", [1, 2 * BPC], F32, isOutput=True)

    with tile.TileContext(nc) as tc:
        with (
            tc.tile_pool(name="w", bufs=1) as wpool,
            tc.tile_pool(name="state", bufs=1) as spool,
            tc.tile_pool(name="grp", bufs=2) as gpool,
            tc.tile_pool(name="ps", bufs=2, space="PSUM") as pspool,
        ):
            aw = wpool.tile([DA, 2 * DEPTH * DA], MM_DT, tag="aw")
            bw = wpool.tile([DB, 2 * DEPTH * DB], MM_DT, tag="bw")
            gw = wpool.tile([DA, 2 * DA], MM_DT, tag="gw")
            ut = wpool.tile([DA, BPC], MM_DT, tag="ut")
            vt = wpool.tile([DB, BPC], MM_DT, tag="vt")
            ones = wpool.tile([64, 1], F32, tag="ones")

            for i in range(2 * DEPTH):
                nc.sync.dma_start(out=aw[:, i * DA:(i + 1) * DA], in_=at_d[i])
                nc.sync.dma_start(out=bw[:, i * DB:(i + 1) * DB], in_=bt_d[i])
            nc.sync.dma_start(out=gw[:, 0:DA], in_=g_d[0])
            nc.sync.dma_start(out=gw[:, DA:2 * DA], in_=g_d[1])
            nc.sync.dma_start(out=ut[:], in_=ut_d[:, :])
            nc.sync.dma_start(out=vt[:], in_=vt_d[:, :])
            nc.vector.memset(ones[:], 1.0)

            Ltmp = spool.tile([DA, 32 * BPC], MM_DT, tag="Ltmp")
            Lbuf = spool.tile([DA, 64 * BPC], MM_DT, tag="Lbuf")
            Rtmp = spool.tile([DB, 32 * BPC], MM_DT, tag="Rtmp")
            Rbuf = spool.tile([DB, 64 * BPC], MM_DT, tag="Rbuf")
            Pbuf = spool.tile([DA, 2 * 64 * BPC], MM_DT, tag="Pbuf")
            Z = spool.tile([64, 2 * BPC], F32, tag="Z")
            esb = spool.tile([1, 2 * BPC], F32, tag="esb")

            def recursion(w_tile, init_ap, buf_tmp, buf_big):
                cur, n_in = init_ap, BPC
                for k in range(DEPTH):
                    dst = buf_tmp if k % 2 == 0 else buf_big
                    for p in range(2):
                        lhsT = w_tile[:, (2 * k + p) * 128:(2 * k + p + 1) * 128]
                        for c0 in range(0, n_in, 512):
                            cw = min(512, n_in - c0)
                            ps = pspool.tile([128, 512], F32, tag="mm")
                            nc.tensor.matmul(
                                ps[:, :cw], lhsT, cur[:, c0:c0 + cw],
                                start=True, stop=True,
                            )
                            nc.vector.tensor_copy(
                                dst[:, p * n_in + c0:p * n_in + c0 + cw],
                                ps[:, :cw],
                            )
                    cur, n_in = dst[:, :2 * n_in], 2 * n_in
                return cur  # (128, 64*BPC)

            Lfin = recursion(aw, ut[:], Ltmp, Lbuf)
            Rfin = recursion(bw, vt[:], Rtmp, Rbuf)

            # P = [G0 @ L | G1 @ L]  -> (128, (q, j, b))
            NL = 64 * BPC  # 4096
            for q in range(2):
                for c0 in range(0, NL, 512):
                    ps = pspool.tile([128, 512], F32, tag="mm")
                    nc.tensor.matmul(
                        ps[:], gw[:, q * DA:(q + 1) * DA], Lfin[:, c0:c0 + 512],
                        start=True, stop=True,
                    )
                    nc.vector.tensor_copy(Pbuf[:, q * NL + c0:q * NL + c0 + 512], ps[:])

            Lr = Lfin.rearrange("p (j b) -> p j b", b=BPC)
            Rr = Rfin.rearrange("p (j b) -> p j b", b=BPC)
            Pr = Pbuf[:].rearrange("p (q j b) -> p q j b", q=2, b=BPC)

            # Per-batch quadratic forms, 8 batches per group.
            GRP = 8
            n_groups = BPC // GRP
            for g in range(n_groups):
                slg = pspool.tile([64, GRP * 128], F32, tag="slg")
                srg = pspool.tile([64, GRP * 64], F32, tag="srg")
                for i in range(GRP):
                    b = g * GRP + i
                    nc.tensor.matmul(
                        slg[:, i * 128:(i + 1) * 128], Lr[:, :, b], Pr[:, :, :, b],
                        start=True, stop=True,
                    )
                    nc.tensor.matmul(
                        srg[:, i * 64:(i + 1) * 64], Rr[:, :, b], Rr[:, :, b],
                        start=True, stop=True,
                    )
                srsb = gpool.tile([64, GRP * 64], F32, tag="srsb")
                nc.vector.tensor_copy(srsb[:], srg[:])
                slg_r = slg[:].rearrange("p (i q j) -> p i q j", q=2, j=64)
                srsb_r = srsb[:].rearrange("p (i j) -> p i j", j=64)
                t0 = gpool.tile([64, GRP * 64], F32, tag="t0")
                t1 = gpool.tile([64, GRP * 64], F32, tag="t1")
                t0_r = t0[:].rearrange("p (i j) -> p i j", j=64)
                t1_r = t1[:].rearrange("p (i j) -> p i j", j=64)
                nc.vector.tensor_mul(t0_r, slg_r[:, :, 0, :], srsb_r)
                nc.vector.tensor_mul(t1_r, slg_r[:, :, 1, :], srsb_r)
                nc.vector.reduce_sum(
                    out=Z[:, g * GRP:(g + 1) * GRP], in_=t0_r,
                    axis=mybir.AxisListType.X,
                )
                nc.vector.reduce_sum(
                    out=Z[:, BPC + g * GRP:BPC + (g + 1) * GRP], in_=t1_r,
                    axis=mybir.AxisListType.X,
                )

            # e[q*BPC + b] = sum over the 64 term-partitions
            zps = pspool.tile([1, 2 * BPC], F32, tag="mm")
            nc.tensor.matmul(zps[:], ones[:], Z[:], start=True, stop=True)
            nc.vector.tensor_copy(esb[:], zps[:])
            nc.sync.dma_start(out=out_d[:, :], in_=esb[:])

    nc.compile()
    return nc


def _get_nc():
    if "nc" not in _nc_cache:
        _nc_cache["nc"] = _build_nc()
    return _nc_cache["nc"]


# ----------------------------------------------------------------------------
# Entry point
# ----------------------------------------------------------------------------

def kernel(X, theta, A, B, D, _trace=False):
    U, V, AT, BT, G = _host_prep(X, theta, A, B, D)
    np_mm = np.float32 if MM_DT == mybir.dt.float32 else mybir.dt.np(MM_DT)
    at = np.ascontiguousarray(AT, dtype=np_mm)
    bt = np.ascontiguousarray(BT, dtype=np_mm)
    g = np.ascontiguousarray(G, dtype=np_mm)
    in_maps = []
    for i in range(N_CORES):
        sl = slice(i * BPC, (i + 1) * BPC)
        in_maps.append(
            {
                "ut": np.ascontiguousarray(U[sl].T, dtype=np_mm),
                "vt": np.ascontiguousarray(V[sl].T, dtype=np_mm),
                "at": at,
                "bt": bt,
                "g": g,
            }
        )
    nc = _get_nc()
    res = run_bass_kernel_spmd(nc, in_maps, list(range(N_CORES)), trace=_trace)
    outs = []
    for i in range(N_CORES):
        e = res.results[i]["out"].reshape(2, BPC).T  # (64, 2)
        outs.append(e)
    full = np.concatenate(outs, axis=0).astype(np.float32)
    if _trace:
        _nc_cache["last_exec_ns"] = res.exec_time_ns
        _nc_cache["last_results"] = res
    return full
